# revision 8
# baseline (speedup 1.0000x reference)
"""Trainium2 Bass kernel for nn_DeepMemoryUnit (scatter_memory).

Strategy (8 NeuronCores, single SPMD launch):
  - Banked linears (W_read / W_wq / W_merge / W_ws) are expert-parallel: each
    core owns 2 of the 16 banks and computes partial sums over its banks for
    ALL batch rows; partials are combined with small on-chip collectives
    (AllReduce / AllGather / ReduceScatter, <=128KB each).
  - The memory tensor (32,8,2048,64) is data-parallel over batch: each core
    owns 4 batch rows (16.8 MB fp32), keeps them SBUF-resident across the
    read (express) and write (store) phases, and writes its shard of
    new_memories.
  - Scores (contraction over d=64) run on the TensorEngine from a bf16
    host-pretransposed copy of memories (2 heads packed per 128 partitions);
    softmax normalization is deferred (read = (sum_m e_m mem_m) / sum_m e_m),
    so only O(64) values are ever normalized.
  - The rank-1 store update (mem + w (x) st) uses two DVE tensor_tensor
    passes per (batch, head) slice with 0-stride broadcast APs.

Host-side prep (cheap, index-dependent): scatter sel_probs into a dense
(32,16) bank-coefficient matrix, fold the 1/sqrt(64) score scale into
W_read/W_wq, compute effective bias rows, transpose query and memories.
"""

import contextlib
import ctypes
import sys
import types

import numpy as np
import ml_dtypes

import concourse.bass as bass
import concourse.bacc as bacc
import concourse.tile as tile
from concourse import mybir
from concourse._compat import with_exitstack

F32 = mybir.dt.float32
BF16 = mybir.dt.bfloat16
AX = mybir.AxisListType
OP = mybir.AluOpType
AF = mybir.ActivationFunctionType
BF16_NP = ml_dtypes.bfloat16

N_CORES = 8
B, D, H, DM, M, HD = 32, 1024, 8, 64, 2048, 512
BL = B // N_CORES            # 4 local batches per core
T = M // 128                 # 16 m-chunks per slice
NPAIR = H // 2               # 4 head-pairs per batch
GROUPS = 2                   # pipeline groups (2 local batches each)
GB = BL // GROUPS            # batches per group (2)
GROWS = GB * N_CORES         # rows per group across cores (16)

_CACHE = {}


# --------------------------------------------------------------------------
# device program
# --------------------------------------------------------------------------

def _declare(nc):
    t = {}
    def inp(name, shape, dt):
        t[name] = nc.dram_tensor(name, list(shape), dt, kind="ExternalInput").ap()
    def out(name, shape, dt):
        t[name] = nc.dram_tensor(name, list(shape), dt, kind="ExternalOutput").ap()
    inp("mem", (BL, H, M, DM), F32)
    inp("memt", (BL, H, DM, M), BF16)
    inp("query", (B, D), F32)
    inp("queryT", (D, B), F32)
    inp("conT", (2, B), F32)
    inp("wr", (2, D, HD), BF16)
    inp("wwq", (2, D, HD), BF16)
    inp("wm", (2, HD, D), BF16)
    inp("wws", (2, D, HD), BF16)
    inp("b_qwq", (B, 2 * HD), F32)
    inp("b_m", (B, D), F32)
    inp("b_s", (B, HD), F32)
    inp("lnsc", (1, D), F32)
    inp("lnbi", (1, D), F32)
    inp("ident", (128, 128), F32)
    inp("ones", (128, 1), F32)
    out("resp", (B, D), F32)
    out("newmem", (BL, H, M, DM), F32)
    # collective scratch (internal DRAM)
    t["ar1_in"] = nc.dram_tensor("ar1_in", [B, 2 * HD], F32).ap()
    t["ar1_out"] = nc.dram_tensor("ar1_out", [BL, 2 * HD], F32).ap()
    for g in range(GROUPS):
        t[f"rh_in{g}"] = nc.dram_tensor(f"rh_in{g}", [GB, HD], F32).ap()
        t[f"rh_out{g}"] = nc.dram_tensor(f"rh_out{g}", [GROWS, HD], F32, addr_space="Shared").ap()
        t[f"mg_in{g}"] = nc.dram_tensor(f"mg_in{g}", [GROWS, D], F32).ap()
        t[f"mg_out{g}"] = nc.dram_tensor(f"mg_out{g}", [GROWS, D], F32, addr_space="Shared").ap()
        t[f"sg_in{g}"] = nc.dram_tensor(f"sg_in{g}", [GROWS, HD], F32).ap()
        t[f"sg_out{g}"] = nc.dram_tensor(f"sg_out{g}", [GB, HD], F32).ap()
    return t


@with_exitstack
def _emit(ctx, tc, t):
    nc = tc.nc
    RG = [list(range(N_CORES))]
    cc_sem = nc.alloc_semaphore("cc_sem")
    cc_count = [0]

    def collective(kind, in_ap, out_ap):
        with tc.tile_critical():
            op = OP.bypass if kind == "AllGather" else OP.add
            nc.gpsimd.collective_compute(
                kind, op, ins=[in_ap], outs=[out_ap], replica_groups=RG,
            ).then_inc(cc_sem)
            cc_count[0] += 1
            nc.gpsimd.wait_ge(cc_sem, cc_count[0])

    const = ctx.enter_context(tc.tile_pool(name="const", bufs=1))
    memp = ctx.enter_context(tc.tile_pool(name="memp", bufs=1))
    memtp = ctx.enter_context(tc.tile_pool(name="memtp", bufs=2))
    wp = ctx.enter_context(tc.tile_pool(name="wp", bufs=4))
    ep = ctx.enter_context(tc.tile_pool(name="ep", bufs=1))
    small = ctx.enter_context(tc.tile_pool(name="small", bufs=2))
    stage = ctx.enter_context(tc.tile_pool(name="stage", bufs=1))
    scat = ctx.enter_context(tc.tile_pool(name="scat", bufs=2))

    ps_lin = ctx.enter_context(tc.tile_pool(name="ps_lin", bufs=1, space="PSUM"))
    ps_s = ctx.enter_context(tc.tile_pool(name="ps_s", bufs=2, space="PSUM"))
    ps_r = ctx.enter_context(tc.tile_pool(name="ps_r", bufs=1, space="PSUM"))
    ps_t = ctx.enter_context(tc.tile_pool(name="ps_t", bufs=2, space="PSUM"))

    # ---------------- constants ----------------
    ident = const.tile([128, 128], F32)
    nc.sync.dma_start(ident[:], t["ident"][:])
    ones = const.tile([128, 1], F32)
    nc.sync.dma_start(ones[:], t["ones"][:])
    lnsc_r = const.tile([GROWS, D], F32)
    lnbi_r = const.tile([GROWS, D], F32)
    lnsc_1 = stage.tile([1, D], F32, tag="g16", bufs=3)
    lnbi_1 = stage.tile([1, D], F32, tag="g16", bufs=3)
    nc.sync.dma_start(lnsc_1[:], t["lnsc"][:])
    nc.sync.dma_start(lnbi_1[:], t["lnbi"][:])
    nc.gpsimd.partition_broadcast(lnsc_r[:], lnsc_1[:])
    nc.gpsimd.partition_broadcast(lnbi_r[:], lnbi_1[:])
    crep = []
    for e in range(2):
        c1 = const.tile([1, B], F32, tag=f"con1_{e}")
        nc.sync.dma_start(c1[:], t["conT"][e:e + 1, :])
        r = const.tile([128, B], F32, tag=f"crep{e}")
        nc.gpsimd.partition_broadcast(r[:], c1[:])
        crep.append(r)

    # ---------------- resident memories (fp32, natural layout) -------------
    # mem_sl[(bl,h)][p, t*DM+d] = mem[bl, h, t*128+p, d]
    mem_sl = {}
    for bl in range(BL):
        for h in range(H):
            ms = memp.tile([128, T * DM], F32, tag=f"mem_{bl}_{h}")
            nc.sync.dma_start(
                ms[:].rearrange("p (tt d) -> p tt d", tt=T),
                t["mem"][bl, h].rearrange("(tt p) d -> p tt d", p=128))
            mem_sl[(bl, h)] = ms

    # ---------------- phase 1: q|wq banked linear + AR1 ----------------
    qT = stage.tile([128, 8 * B], F32, tag="g16", bufs=3)  # (p, kc, b)
    nc.sync.dma_start(
        qT[:].rearrange("p (k b) -> p k b", k=8),
        t["queryT"][:].rearrange("(k p) b -> p k b", p=128),
    )
    xet = []
    for e in range(2):
        xe = const.tile([128, 8 * B], BF16, tag=f"xet{e}")
        nc.vector.tensor_tensor(
            out=xe[:].rearrange("p (k b) -> p k b", k=8),
            in0=qT[:].rearrange("p (k b) -> p k b", k=8),
            in1=crep[e][:].rearrange("p (o b) -> p o b", o=1).broadcast_to((128, 8, B)),
            op=OP.mult,
        )
        xet.append(xe)

    psq = ps_lin.tile([B, 2 * HD], F32, tag="pslin")
    for wi, wname in enumerate(("wr", "wwq")):
        for e in range(2):
            for kc in range(8):
                wt = wp.tile([128, HD], BF16, tag="wchunk")
                nc.sync.dma_start(wt[:], t[wname][e, kc * 128:(kc + 1) * 128, :])
                nc.tensor.matmul(
                    psq[:, wi * HD:(wi + 1) * HD],
                    xet[e][:, kc * B:(kc + 1) * B],
                    wt[:],
                    start=(e == 0 and kc == 0),
                    stop=(e == 1 and kc == 7),
                )
    bq = stage.tile([B, 2 * HD], F32, tag="g16", bufs=3)
    nc.sync.dma_start(bq[:], t["b_qwq"][:])
    qwq_st = stage.tile([B, 2 * HD], F32, tag="g16", bufs=3)
    nc.vector.tensor_tensor(out=qwq_st[:], in0=psq[:], in1=bq[:], op=OP.add)
    nc.sync.dma_start(t["ar1_in"][:], qwq_st[:])
    collective("ReduceScatter", t["ar1_in"][:], t["ar1_out"][:])
    qwq = stage.tile([BL, 2 * HD], F32, tag="g16", bufs=3)
    nc.sync.dma_start(qwq[:], t["ar1_out"][:])

    # qwqT[p, kc*BL+bl] = qwq[bl, kc*128+p]  (bf16, local batches only)
    pst = ps_t.tile([128, 8 * BL], F32, tag="pst")
    for kc in range(8):
        nc.tensor.transpose(
            pst[:, kc * BL:(kc + 1) * BL], qwq[:, kc * 128:(kc + 1) * 128],
            ident[0:BL, 0:BL])
    qwqT = const.tile([128, 8 * BL], BF16)
    nc.vector.tensor_copy(qwqT[:], pst[:])

    # block-diagonal per-pair score weights (128, 4): cols q_e, q_o, wq_e, wq_o
    qw4 = {}
    for bl in range(BL):
        for j in range(NPAIR):
            w4 = const.tile([128, 4], BF16, tag=f"qw4_{bl}_{j}")
            nc.gpsimd.memset(w4[:], 0.0)
            # cols {0,2} rows 0-63 <- qwqT[0:64, {j, 4+j}*BL + bl]
            nc.vector.tensor_copy(
                w4[0:64, :].rearrange("p (a c) -> p a c", a=2)[:, :, 0],
                qwqT[0:64, j * BL + bl:j * BL + bl + 4 * BL + 1:4 * BL],
            )
            nc.vector.tensor_copy(
                w4[64:128, :].rearrange("p (a c) -> p a c", a=2)[:, :, 1],
                qwqT[64:128, j * BL + bl:j * BL + bl + 4 * BL + 1:4 * BL],
            )
            qw4[(bl, j)] = w4

    # ---------------- express phase (scores, exp, readsum) ----------------
    inv_g, e_nat = {}, {}

    def express_group(g):
        sums = small.tile([128, GB * NPAIR * 4], F32, tag=f"sums{g}")
        psr = ps_r.tile([1, GROWS * DM], F32, tag="psread")
        for i in range(GB):
            bl = g * GB + i
            for j in range(NPAIR):
                mt = memtp.tile([128, M], BF16, tag="memt")
                nc.sync.dma_start(mt[0:64, :], t["memt"][bl, 2 * j])
                nc.sync.dma_start(mt[64:128, :], t["memt"][bl, 2 * j + 1])
                pss = ps_s.tile([128, T * 4], F32, tag="pss")
                for mc in range(T):
                    nc.tensor.matmul(
                        pss[:, mc * 4:(mc + 1) * 4],
                        mt[:, mc * 128:(mc + 1) * 128],
                        qw4[(bl, j)][:],
                        start=True, stop=True,
                    )
                en = ep.tile([128, T * 4], F32, tag=f"e_{bl}_{j}")
                nc.scalar.activation(en[:], pss[:], AF.Exp)
                e_nat[(bl, j)] = en
                pg = i * NPAIR + j
                nc.vector.tensor_reduce(
                    out=sums[:, pg * 4:(pg + 1) * 4],
                    in_=en[:].rearrange("p (tt k) -> p k tt", k=4),
                    axis=AX.X, op=OP.add,
                )
                for par in range(2):
                    h = 2 * j + par
                    s = i * H + h
                    msl = mem_sl[(bl, h)]
                    for tt in range(T):
                        nc.tensor.matmul(
                            psr[0:1, s * DM:(s + 1) * DM],
                            en[:, 4 * tt + par:4 * tt + par + 1],
                            msl[:, tt * DM:(tt + 1) * DM],
                            start=(tt == 0), stop=(tt == T - 1),
                        )
        # softmax denominators: cross-partition sum via ones-matmul
        psum1 = ps_t.tile([1, GB * NPAIR * 4], F32, tag="pst")
        nc.tensor.matmul(psum1[:], ones[:], sums[:], start=True, stop=True)
        inv = small.tile([1, GB * NPAIR * 4], F32, tag=f"inv{g}")
        nc.vector.reciprocal(inv[:], psum1[:])
        inv_g[g] = inv
        # normalized read heads -> DRAM -> AllGather
        rhst = stage.tile([1, GROWS * DM], F32, tag="rhst", bufs=1)
        for i in range(GB):
            for h in range(H):
                s = i * H + h
                pg = i * NPAIR + h // 2
                nc.scalar.activation(
                    rhst[0:1, s * DM:(s + 1) * DM],
                    psr[0:1, s * DM:(s + 1) * DM],
                    AF.Copy,
                    scale=inv[0:1, pg * 4 + (h % 2):pg * 4 + (h % 2) + 1],
                )
        nc.sync.dma_start(t[f"rh_in{g}"][:], rhst[:])
        collective("AllGather", t[f"rh_in{g}"][:], t[f"rh_out{g}"][:])

    # ---------------- merge + LN + st linear per group ----------------
    def tail_group(g):
        rh = stage.tile([GROWS, HD], F32, tag="rh_full", bufs=1)
        nc.sync.dma_start(rh[:], t[f"rh_out{g}"][:])
        # rhT[p, kc*16+r] = rh[r, kc*128+p], bank-scaled bf16
        pstr = ps_t.tile([128, 4 * GROWS], F32, tag="pst")
        for kc in range(4):
            nc.tensor.transpose(
                pstr[:, kc * GROWS:(kc + 1) * GROWS],
                rh[:, kc * 128:(kc + 1) * 128], ident[0:GROWS, 0:GROWS])
        # group coefficient rows: conT cols {4r + 2g + i}
        cgr = []
        for e in range(2):
            cg1 = small.tile([1, GROWS], F32, tag=f"cg1_{e}")
            nc.sync.dma_start(
                cg1[:],
                t["conT"][e:e + 1, :].rearrange("o (r i) -> o r i", i=BL)[:, :, 2 * g:2 * g + 2],
            )
            r = small.tile([128, GROWS], F32, tag=f"cgr{e}")
            nc.gpsimd.partition_broadcast(r[:], cg1[:])
            cgr.append(r)
        rhT_s = []
        for e in range(2):
            x = stage.tile([128, 4 * GROWS], BF16, tag="bsmall", bufs=4)
            nc.vector.tensor_tensor(
                out=x[:].rearrange("p (k r) -> p k r", k=4),
                in0=pstr[:].rearrange("p (k r) -> p k r", k=4),
                in1=cgr[e][:].rearrange("p (o r) -> p o r", o=1).broadcast_to((128, 4, GROWS)),
                op=OP.mult,
            )
            rhT_s.append(x)
        psm = ps_lin.tile([GROWS, D], F32, tag="pslin")
        for half in range(2):
            for e in range(2):
                for kc in range(4):
                    wt = wp.tile([128, HD], BF16, tag="wchunk")
                    nc.sync.dma_start(
                        wt[:], t["wm"][e, kc * 128:(kc + 1) * 128,
                                       half * HD:(half + 1) * HD])
                    nc.tensor.matmul(
                        psm[:, half * HD:(half + 1) * HD],
                        rhT_s[e][:, kc * GROWS:(kc + 1) * GROWS],
                        wt[:],
                        start=(e == 0 and kc == 0), stop=(e == 1 and kc == 3),
                    )
        bm = stage.tile([GROWS, D], F32, tag="g16", bufs=3)
        nc.sync.dma_start(
            bm[:], t["b_m"][:].rearrange("(r i) d -> r i d", i=BL)[:, 2 * g:2 * g + 2, :])
        mst = stage.tile([GROWS, D], F32, tag="g16", bufs=3)
        nc.vector.tensor_tensor(out=mst[:], in0=psm[:], in1=bm[:], op=OP.add)
        nc.sync.dma_start(t[f"mg_in{g}"][:], mst[:])
        collective("AllReduce", t[f"mg_in{g}"][:], t[f"mg_out{g}"][:])

        x = stage.tile([GROWS, D], F32, tag="respg", bufs=1)
        nc.sync.dma_start(x[:], t[f"mg_out{g}"][:])
        qg = stage.tile([GROWS, D], F32, tag="g16", bufs=3)
        nc.sync.dma_start(
            qg[:], t["query"][:].rearrange("(r i) d -> r i d", i=BL)[:, 2 * g:2 * g + 2, :])
        # layernorm(x + qg)
        nc.vector.tensor_tensor(out=x[:], in0=x[:], in1=qg[:], op=OP.add)
        mu = small.tile([GROWS, 1], F32, tag="mu")
        nc.vector.tensor_reduce(out=mu[:], in_=x[:], axis=AX.X, op=OP.add)
        nc.vector.tensor_scalar(out=mu[:], in0=mu[:], scalar1=1.0 / D, scalar2=None,
                                op0=OP.mult)
        nc.vector.tensor_scalar(out=x[:], in0=x[:], scalar1=mu[:], scalar2=None,
                                op0=OP.subtract)
        sq = stage.tile([GROWS, D], F32, tag="g16", bufs=3)
        ssq = small.tile([GROWS, 1], F32, tag="ssq")
        nc.scalar.activation(sq[:], x[:], AF.Square, accum_out=ssq[:])
        nc.vector.tensor_scalar(out=ssq[:], in0=ssq[:], scalar1=float(D) * 1e-5,
                                scalar2=None, op0=OP.add)
        sstd = small.tile([GROWS, 1], F32, tag="sstd")
        nc.scalar.activation(sstd[:], ssq[:], AF.Sqrt)
        rstd = small.tile([GROWS, 1], F32, tag="rstd")
        nc.vector.reciprocal(rstd[:], sstd[:])
        nc.vector.tensor_scalar(out=x[:], in0=x[:], scalar1=rstd[:],
                                scalar2=float(np.sqrt(D)), op0=OP.mult, op1=OP.mult)
        nc.vector.tensor_tensor(out=x[:], in0=x[:], in1=lnsc_r[:], op=OP.mult)
        nc.vector.tensor_tensor(out=x[:], in0=x[:], in1=lnbi_r[:], op=OP.add)
        nc.sync.dma_start(
            t["resp"][:].rearrange("(r i) d -> r i d", i=BL)[:, 2 * g:2 * g + 2, :], x[:])

        # st linear: stT[p, kc*16+r] = x[r, kc*128+p]
        psr2 = ps_t.tile([128, 8 * GROWS], F32, tag="pst")
        for kc in range(8):
            nc.tensor.transpose(
                psr2[:, kc * GROWS:(kc + 1) * GROWS],
                x[:, kc * 128:(kc + 1) * 128], ident[0:GROWS, 0:GROWS])
        stT_s = []
        for e in range(2):
            y = stage.tile([128, 8 * GROWS], BF16, tag="bsmall", bufs=4)
            nc.vector.tensor_tensor(
                out=y[:].rearrange("p (k r) -> p k r", k=8),
                in0=psr2[:].rearrange("p (k r) -> p k r", k=8),
                in1=cgr[e][:].rearrange("p (o r) -> p o r", o=1).broadcast_to((128, 8, GROWS)),
                op=OP.mult,
            )
            stT_s.append(y)
        pss2 = ps_lin.tile([GROWS, HD], F32, tag="pslin")
        for e in range(2):
            for kc in range(8):
                wt = wp.tile([128, HD], BF16, tag="wchunk")
                nc.sync.dma_start(wt[:], t["wws"][e, kc * 128:(kc + 1) * 128, :])
                nc.tensor.matmul(
                    pss2[:],
                    stT_s[e][:, kc * GROWS:(kc + 1) * GROWS],
                    wt[:],
                    start=(e == 0 and kc == 0), stop=(e == 1 and kc == 7),
                )
        bs = stage.tile([GROWS, HD], F32, tag="bs_sst", bufs=2)
        nc.sync.dma_start(
            bs[:], t["b_s"][:].rearrange("(r i) d -> r i d", i=BL)[:, 2 * g:2 * g + 2, :])
        sst = stage.tile([GROWS, HD], F32, tag="bs_sst", bufs=2)
        nc.vector.tensor_tensor(out=sst[:], in0=pss2[:], in1=bs[:], op=OP.add)
        nc.sync.dma_start(t[f"sg_in{g}"][:], sst[:])
        collective("ReduceScatter", t[f"sg_in{g}"][:], t[f"sg_out{g}"][:])

    # ---------------- store phase (scatter-add + writeback) ----------------
    def scatter_group(g):
        for i in range(GB):
            bl = g * GB + i
            strow = small.tile([1, HD], F32, tag="strow")
            nc.sync.dma_start(strow[:], t[f"sg_out{g}"][i])
            for h in range(H):
                j, par = h // 2, h % 2
                pg = i * NPAIR + j
                strep = scat.tile([128, DM], F32, tag="strep")
                nc.gpsimd.partition_broadcast(strep[:], strow[0:1, h * DM:(h + 1) * DM])
                iwq = scat.tile([128, 1], F32, tag="iwq")
                nc.gpsimd.partition_broadcast(
                    iwq[:], inv_g[g][0:1, pg * 4 + 2 + par:pg * 4 + 3 + par])
                nc.vector.tensor_scalar(out=strep[:], in0=strep[:], scalar1=iwq[:],
                                        scalar2=None, op0=OP.mult)
                en = e_nat[(bl, j)]
                tmp = scat.tile([128, T * DM], F32, tag="tmp")
                nc.vector.tensor_tensor(
                    out=tmp[:].rearrange("p (tt d) -> p tt d", tt=T),
                    in0=en[:].rearrange("p (tt k) -> p tt k", tt=T)[:, :, 2 + par:3 + par]
                        .broadcast_to((128, T, DM)),
                    in1=strep[:].rearrange("p (o d) -> p o d", o=1)
                        .broadcast_to((128, T, DM)),
                    op=OP.mult,
                )
                nc.vector.tensor_tensor(
                    out=tmp[:], in0=mem_sl[(bl, h)][:], in1=tmp[:], op=OP.add)
                nc.sync.dma_start(
                    t["newmem"][bl, h].rearrange("(tt p) d -> p tt d", p=128),
                    tmp[:].rearrange("p (tt d) -> p tt d", tt=T),
                )

    # ---------------- schedule ----------------
    express_group(0)
    express_group(1)
    tail_group(0)
    tail_group(1)
    scatter_group(0)
    scatter_group(1)


def _build():
    if "nc" in _CACHE:
        return _CACHE["nc"], _CACHE["t"]
    nc = bacc.Bacc("TRN2", target_bir_lowering=False, debug=False,
                   num_devices=N_CORES)
    t = _declare(nc)
    with tile.TileContext(nc) as tc:
        _emit(tc, t)
    nc.compile()
    _CACHE["nc"] = nc
    _CACHE["t"] = t
    return nc, t


# --------------------------------------------------------------------------
# host side
# --------------------------------------------------------------------------

def _prep_in_maps(inputs):
    mem = np.asarray(inputs["memories"], dtype=np.float32)
    query = np.asarray(inputs["query"], dtype=np.float32)
    sel = np.asarray(inputs["sel_index"])
    probs = np.asarray(inputs["sel_probs"], dtype=np.float32)

    c = np.zeros((B, 16), np.float32)
    for k in range(sel.shape[1]):
        np.add.at(c, (np.arange(B), sel[:, k]), probs[:, k])

    memt = np.ascontiguousarray(mem.transpose(0, 1, 3, 2)).astype(BF16_NP)
    wr = (np.asarray(inputs["W_read"], np.float32) * 0.125).astype(BF16_NP)
    wwq = (np.asarray(inputs["W_wq"], np.float32) * 0.125).astype(BF16_NP)
    wm = np.asarray(inputs["W_merge"], np.float32).astype(BF16_NP)
    wws = np.asarray(inputs["W_ws"], np.float32).astype(BF16_NP)

    b_qwq = np.concatenate([
        (c @ np.asarray(inputs["b_read"], np.float32)) * 0.125,
        (c @ np.asarray(inputs["b_wq"], np.float32)) * 0.125,
    ], axis=1).astype(np.float32)
    b_m = (c @ np.asarray(inputs["b_merge"], np.float32)).astype(np.float32)
    b_s = (c @ np.asarray(inputs["b_ws"], np.float32)).astype(np.float32)
    zq = np.zeros_like(b_qwq)
    zm = np.zeros_like(b_m)
    zs = np.zeros_like(b_s)

    queryT = np.ascontiguousarray(query.T)
    lnsc = np.asarray(inputs["ln_scale"], np.float32).reshape(1, D)
    lnbi = np.asarray(inputs["ln_bias"], np.float32).reshape(1, D)
    ident = np.eye(128, dtype=np.float32)
    ones = np.ones((128, 1), dtype=np.float32)

    in_maps = []
    for core in range(N_CORES):
        bs_ = slice(BL * core, BL * (core + 1))
        bk = slice(2 * core, 2 * core + 2)
        in_maps.append({
            "mem": np.ascontiguousarray(mem[bs_]),
            "memt": np.ascontiguousarray(memt[bs_]),
            "query": query,
            "queryT": queryT,
            "conT": np.ascontiguousarray(c[:, bk].T),
            "wr": np.ascontiguousarray(wr[bk]),
            "wwq": np.ascontiguousarray(wwq[bk]),
            "wm": np.ascontiguousarray(wm[bk]),
            "wws": np.ascontiguousarray(wws[bk]),
            "b_qwq": b_qwq if core == 0 else zq,
            "b_m": b_m if core == 0 else zm,
            "b_s": b_s if core == 0 else zs,
            "lnsc": lnsc,
            "lnbi": lnbi,
            "ident": ident,
            "ones": ones,
        })
    return in_maps


def _assemble(results):
    response = results[0]["resp"]
    new_mem = np.concatenate([results[c]["newmem"] for c in range(N_CORES)], axis=0)
    return response, new_mem


def kernel(**inputs):
    from concourse.bass_utils import run_bass_kernel_spmd
    nc, _ = _build()
    in_maps = _prep_in_maps(inputs)
    res = run_bass_kernel_spmd(nc, in_maps, list(range(N_CORES)))
    return _assemble(res.results)


# --------------------------------------------------------------------------
# profiling helper (not used by the grading path)
# --------------------------------------------------------------------------

def _register_ntff_hook():
    if "antenv.axon_hooks" in sys.modules:
        return
    holder = {"h": None}
    mod = types.ModuleType("antenv.axon_hooks")
    mod.set_axon_ntff_profile_hook = lambda h: holder.__setitem__("h", h)
    mod.get_axon_ntff_profile_hook = lambda: holder["h"]
    sys.modules["antenv.axon_hooks"] = mod
    try:
        lib = ctypes.CDLL("/opt/axon/libaxon_pjrt.so")
        lib.axon_start_nrt_profile.argtypes = [ctypes.POINTER(ctypes.c_int64), ctypes.c_size_t]
        lib.axon_start_nrt_profile.restype = ctypes.c_int64
        lib.axon_stop_nrt_profile.argtypes = [ctypes.c_char_p]
        lib.axon_stop_nrt_profile.restype = ctypes.c_int64
    except OSError:
        return

    @contextlib.contextmanager
    def _hook(output_dir, device_ids):
        import jax
        jax.devices()
        if device_ids:
            ids = (ctypes.c_int64 * len(device_ids))(*device_ids)
            rc = lib.axon_start_nrt_profile(ids, len(device_ids))
        else:
            rc = lib.axon_start_nrt_profile(None, 0)
        if rc != 0:
            raise RuntimeError(f"axon_start_nrt_profile rc={rc}")
        try:
            yield
        finally:
            n = lib.axon_stop_nrt_profile(str(output_dir).encode())
            print(f"profile: {n} file(s) written to {output_dir}", file=sys.stderr)

    mod.set_axon_ntff_profile_hook(_hook)


def kernel_profiled(tmpdir, **inputs):
    import concourse.bass_utils as bass_utils
    _register_ntff_hook()
    bass_utils.upload_artifacts = lambda d: f"local://{d}"
    nc, _ = _build()
    in_maps = _prep_in_maps(inputs)
    res = bass_utils.run_bass_kernel_spmd(
        nc, in_maps, list(range(N_CORES)), trace=True, tmpdir=tmpdir)
    return _assemble(res.results), res.exec_time_ns


# revision 11
# speedup vs baseline: 1.0327x; 1.0327x over previous
"""Trainium2 Bass kernel for nn_DeepMemoryUnit (scatter_memory).

Strategy (8 NeuronCores, single SPMD launch):
  - Banked linears (W_read / W_wq / W_merge / W_ws) are expert-parallel: each
    core owns 2 of the 16 banks and computes partial sums over its banks for
    ALL batch rows; partials are combined with small on-chip collectives
    (AllReduce / AllGather / ReduceScatter, <=128KB each).
  - The memory tensor (32,8,2048,64) is data-parallel over batch: each core
    owns 4 batch rows (16.8 MB fp32), keeps them SBUF-resident across the
    read (express) and write (store) phases, and writes its shard of
    new_memories.
  - Scores (contraction over d=64) run on the TensorEngine from a bf16
    host-pretransposed copy of memories (2 heads packed per 128 partitions);
    softmax normalization is deferred (read = (sum_m e_m mem_m) / sum_m e_m),
    so only O(64) values are ever normalized.
  - The rank-1 store update (mem + w (x) st) uses two DVE tensor_tensor
    passes per (batch, head) slice with 0-stride broadcast APs.

Host-side prep (cheap, index-dependent): scatter sel_probs into a dense
(32,16) bank-coefficient matrix, fold the 1/sqrt(64) score scale into
W_read/W_wq, compute effective bias rows, transpose query and memories.
"""

import contextlib
import ctypes
import sys
import types

import numpy as np
import ml_dtypes

import concourse.bass as bass
import concourse.bacc as bacc
import concourse.tile as tile
from concourse import mybir
from concourse._compat import with_exitstack

F32 = mybir.dt.float32
BF16 = mybir.dt.bfloat16
AX = mybir.AxisListType
OP = mybir.AluOpType
AF = mybir.ActivationFunctionType
BF16_NP = ml_dtypes.bfloat16

N_CORES = 8
B, D, H, DM, M, HD = 32, 1024, 8, 64, 2048, 512
BL = B // N_CORES            # 4 local batches per core
T = M // 128                 # 16 m-chunks per slice
NPAIR = H // 2               # 4 head-pairs per batch
GROUPS = 4                   # pipeline groups (1 local batch each)
GB = BL // GROUPS            # batches per group (1)
GROWS = GB * N_CORES         # rows per group across cores (8)

_CACHE = {}


# --------------------------------------------------------------------------
# device program
# --------------------------------------------------------------------------

def _declare(nc):
    t = {}
    def inp(name, shape, dt):
        t[name] = nc.dram_tensor(name, list(shape), dt, kind="ExternalInput").ap()
    def out(name, shape, dt):
        t[name] = nc.dram_tensor(name, list(shape), dt, kind="ExternalOutput").ap()
    inp("mem", (BL, H, M, DM), F32)
    inp("memt", (BL, H, DM, M), BF16)
    inp("query", (B, D), F32)
    inp("queryT", (D, B), F32)
    inp("conT", (2, B), F32)
    inp("wr", (2, D, HD), BF16)
    inp("wwq", (2, D, HD), BF16)
    inp("wm", (2, HD, D), BF16)
    inp("wws", (2, D, HD), BF16)
    inp("b_qwq", (B, 2 * HD), F32)
    inp("b_m", (B, D), F32)
    inp("b_s", (B, HD), F32)
    inp("lnsc", (1, D), F32)
    inp("lnbi", (1, D), F32)
    inp("ident", (128, 128), F32)
    inp("ones", (128, 1), F32)
    inp("onesrow", (1, 128), F32)
    out("resp", (B, D), F32)
    out("newmem", (BL, H, M, DM), F32)
    # collective scratch (internal DRAM)
    t["ar1_in"] = nc.dram_tensor("ar1_in", [B, 2 * HD], F32).ap()
    t["ar1_out"] = nc.dram_tensor("ar1_out", [BL, 2 * HD], F32).ap()
    for g in range(GROUPS):
        t[f"rh_in{g}"] = nc.dram_tensor(f"rh_in{g}", [GB, HD], F32).ap()
        t[f"rh_out{g}"] = nc.dram_tensor(f"rh_out{g}", [GROWS, HD], F32, addr_space="Shared").ap()
        t[f"mg_in{g}"] = nc.dram_tensor(f"mg_in{g}", [GROWS, D], F32).ap()
        t[f"mg_out{g}"] = nc.dram_tensor(f"mg_out{g}", [GROWS, D], F32, addr_space="Shared").ap()
        t[f"sg_in{g}"] = nc.dram_tensor(f"sg_in{g}", [GROWS, HD], F32).ap()
        t[f"sg_out{g}"] = nc.dram_tensor(f"sg_out{g}", [GB, HD], F32).ap()
    return t


@with_exitstack
def _emit(ctx, tc, t):
    nc = tc.nc
    RG = [list(range(N_CORES))]
    cc_sem = nc.alloc_semaphore("cc_sem")
    cc_count = [0]

    def collective(kind, in_ap, out_ap):
        with tc.tile_critical():
            op = OP.bypass if kind == "AllGather" else OP.add
            nc.gpsimd.collective_compute(
                kind, op, ins=[in_ap], outs=[out_ap], replica_groups=RG,
            ).then_inc(cc_sem)
            cc_count[0] += 1
            nc.gpsimd.wait_ge(cc_sem, cc_count[0])

    const = ctx.enter_context(tc.tile_pool(name="const", bufs=1))
    memp = ctx.enter_context(tc.tile_pool(name="memp", bufs=1))
    memtp = ctx.enter_context(tc.tile_pool(name="memtp", bufs=2))
    wp = ctx.enter_context(tc.tile_pool(name="wp", bufs=4))
    ep = ctx.enter_context(tc.tile_pool(name="ep", bufs=1))
    small = ctx.enter_context(tc.tile_pool(name="small", bufs=2))
    stage = ctx.enter_context(tc.tile_pool(name="stage", bufs=1))
    scat = ctx.enter_context(tc.tile_pool(name="scat", bufs=2))

    ps_lin = ctx.enter_context(tc.tile_pool(name="ps_lin", bufs=1, space="PSUM"))
    ps_s = ctx.enter_context(tc.tile_pool(name="ps_s", bufs=2, space="PSUM"))
    ps_r = ctx.enter_context(tc.tile_pool(name="ps_r", bufs=1, space="PSUM"))
    ps_t = ctx.enter_context(tc.tile_pool(name="ps_t", bufs=2, space="PSUM"))

    # ---------------- constants ----------------
    ident = const.tile([128, 128], F32)
    nc.sync.dma_start(ident[:], t["ident"][:])
    ones = const.tile([128, 1], F32)
    nc.sync.dma_start(ones[:], t["ones"][:])
    onesrow_t = const.tile([1, 128], F32)
    nc.sync.dma_start(onesrow_t[:], t["onesrow"][:])
    lnsc_r = const.tile([GROWS, D], F32)
    lnbi_r = const.tile([GROWS, D], F32)
    lnsc_1 = stage.tile([1, D], F32, tag="g16", bufs=3)
    lnbi_1 = stage.tile([1, D], F32, tag="g16", bufs=3)
    nc.sync.dma_start(lnsc_1[:], t["lnsc"][:])
    nc.sync.dma_start(lnbi_1[:], t["lnbi"][:])
    nc.gpsimd.partition_broadcast(lnsc_r[:], lnsc_1[:])
    nc.gpsimd.partition_broadcast(lnbi_r[:], lnbi_1[:])
    crep = []
    for e in range(2):
        c1 = const.tile([1, B], F32, tag=f"con1_{e}")
        nc.sync.dma_start(c1[:], t["conT"][e:e + 1, :])
        r = const.tile([128, B], F32, tag=f"crep{e}")
        nc.gpsimd.partition_broadcast(r[:], c1[:])
        crep.append(r)

    # ---------------- resident memories (fp32, natural layout) -------------
    # mem_sl[(bl,h)][p, tt*DM+d] = mem[bl, h, p*T+tt, d]  (DMA'd per group)
    mem_sl = {}

    def load_mem(bl):
        for h in range(H):
            ms = memp.tile([128, T * DM], F32, tag=f"mem_{bl}_{h}",
                           name=f"mem_{bl}_{h}")
            nc.sync.dma_start(
                ms[:].rearrange("p (tt d) -> p tt d", tt=T),
                t["mem"][bl, h].rearrange("(p tt) d -> p tt d", tt=T))
            mem_sl[(bl, h)] = ms

    # ---------------- phase 1: q|wq banked linear + AR1 ----------------
    qT = stage.tile([128, 8 * B], F32, tag="g16", bufs=3)  # (p, kc, b)
    nc.sync.dma_start(
        qT[:].rearrange("p (k b) -> p k b", k=8),
        t["queryT"][:].rearrange("(k p) b -> p k b", p=128),
    )
    xet = []
    for e in range(2):
        xe = const.tile([128, 8 * B], BF16, tag=f"xet{e}")
        nc.vector.tensor_tensor(
            out=xe[:].rearrange("p (k b) -> p k b", k=8),
            in0=qT[:].rearrange("p (k b) -> p k b", k=8),
            in1=crep[e][:].rearrange("p (o b) -> p o b", o=1).broadcast_to((128, 8, B)),
            op=OP.mult,
        )
        xet.append(xe)

    psq = ps_lin.tile([B, 2 * HD], F32, tag="pslin")
    for wi, wname in enumerate(("wr", "wwq")):
        for e in range(2):
            for kc in range(8):
                wt = wp.tile([128, HD], BF16, tag="wchunk")
                nc.sync.dma_start(wt[:], t[wname][e, kc * 128:(kc + 1) * 128, :])
                nc.tensor.matmul(
                    psq[:, wi * HD:(wi + 1) * HD],
                    xet[e][:, kc * B:(kc + 1) * B],
                    wt[:],
                    start=(e == 0 and kc == 0),
                    stop=(e == 1 and kc == 7),
                )
    bq = stage.tile([B, 2 * HD], F32, tag="g16", bufs=3)
    nc.sync.dma_start(bq[:], t["b_qwq"][:])
    qwq_st = stage.tile([B, 2 * HD], F32, tag="g16", bufs=3)
    nc.vector.tensor_tensor(out=qwq_st[:], in0=psq[:], in1=bq[:], op=OP.add)
    nc.sync.dma_start(t["ar1_in"][:], qwq_st[:])
    collective("ReduceScatter", t["ar1_in"][:], t["ar1_out"][:])
    qwq = stage.tile([BL, 2 * HD], F32, tag="g16", bufs=3)
    nc.sync.dma_start(qwq[:], t["ar1_out"][:])

    # qwqT[p, kc*BL+bl] = qwq[bl, kc*128+p]  (bf16, local batches only)
    pst = ps_t.tile([128, 8 * BL], F32, tag="pst")
    for kc in range(8):
        nc.tensor.transpose(
            pst[:, kc * BL:(kc + 1) * BL], qwq[:, kc * 128:(kc + 1) * 128],
            ident[0:BL, 0:BL])
    qwqT = const.tile([128, 8 * BL], BF16)
    nc.vector.tensor_copy(qwqT[:], pst[:])

    # block-diagonal per-pair score weights (128, 4): cols q_e, q_o, wq_e, wq_o
    qw4 = {}
    for bl in range(BL):
        for j in range(NPAIR):
            w4 = const.tile([128, 4], BF16, tag=f"qw4_{bl}_{j}")
            nc.gpsimd.memset(w4[:], 0.0)
            # cols {0,2} rows 0-63 <- qwqT[0:64, {j, 4+j}*BL + bl]
            nc.vector.tensor_copy(
                w4[0:64, :].rearrange("p (a c) -> p a c", a=2)[:, :, 0],
                qwqT[0:64, j * BL + bl:j * BL + bl + 4 * BL + 1:4 * BL],
            )
            nc.vector.tensor_copy(
                w4[64:128, :].rearrange("p (a c) -> p a c", a=2)[:, :, 1],
                qwqT[64:128, j * BL + bl:j * BL + bl + 4 * BL + 1:4 * BL],
            )
            qw4[(bl, j)] = w4

    # ---------------- express phase (scores, exp, readsum) ----------------
    inv_g, e_nat = {}, {}

    def express_group(g):
        bl = g
        load_mem(bl)
        sums = small.tile([128, NPAIR * 4], F32, tag=f"sums{g}", name=f"sums{g}")
        psr = ps_r.tile([1, H * DM], F32, tag="psread", name=f"psread{g}")
        for j in range(NPAIR):
            mt = memtp.tile([128, M], BF16, tag="memt", name=f"memt{g}_{j}")
            nc.sync.dma_start(mt[0:64, :], t["memt"][bl, 2 * j])
            nc.sync.dma_start(mt[64:128, :], t["memt"][bl, 2 * j + 1])
            pss = ps_s.tile([128, T * 4], F32, tag="pss", name=f"pss{g}_{j}")
            for mc in range(T):
                nc.tensor.matmul(
                    pss[:, mc * 4:(mc + 1) * 4],
                    mt[:, mc * 128:(mc + 1) * 128],
                    qw4[(bl, j)][:],
                    start=True, stop=True,
                )
            en = ep.tile([128, T * 4], F32, tag=f"e_{bl}_{j}", name=f"e_{bl}_{j}")
            nc.scalar.activation(en[:], pss[:], AF.Exp)
            e_nat[(bl, j)] = en
            nc.vector.tensor_reduce(
                out=sums[:, j * 4:(j + 1) * 4],
                in_=en[:].rearrange("p (tt k) -> p k tt", k=4),
                axis=AX.X, op=OP.add,
            )
            for par in range(2):
                h = 2 * j + par
                msl = mem_sl[(bl, h)]
                for tt in range(T):
                    nc.tensor.matmul(
                        psr[0:1, h * DM:(h + 1) * DM],
                        en[:, 4 * tt + par:4 * tt + par + 1],
                        msl[:, tt * DM:(tt + 1) * DM],
                        start=(tt == 0), stop=(tt == T - 1),
                    )
        # softmax denominators: cross-partition sum via ones-matmul
        psum1 = ps_t.tile([1, NPAIR * 4], F32, tag="pst", name=f"psum1_{g}")
        nc.tensor.matmul(psum1[:], ones[:], sums[:], start=True, stop=True)
        inv = small.tile([1, NPAIR * 4], F32, tag=f"inv{g}", name=f"inv{g}")
        nc.vector.reciprocal(inv[:], psum1[:])
        inv_g[g] = inv
        # normalized read heads -> DRAM -> AllGather
        rhst = stage.tile([1, H * DM], F32, tag="rhst", bufs=2, name=f"rhst{g}")
        for h in range(H):
            pg = h // 2
            nc.scalar.activation(
                rhst[0:1, h * DM:(h + 1) * DM],
                psr[0:1, h * DM:(h + 1) * DM],
                AF.Copy,
                scale=inv[0:1, pg * 4 + (h % 2):pg * 4 + (h % 2) + 1],
            )
        nc.sync.dma_start(t[f"rh_in{g}"][:], rhst[:])
        collective("AllGather", t[f"rh_in{g}"][:], t[f"rh_out{g}"][:])

    # ---------------- merge + LN + st linear per group ----------------
    def tail_group(g):
        rh = stage.tile([GROWS, HD], F32, tag="rh_full", bufs=1)
        nc.sync.dma_start(rh[:], t[f"rh_out{g}"][:])
        # rhT[p, kc*16+r] = rh[r, kc*128+p], bank-scaled bf16
        pstr = ps_t.tile([128, 4 * GROWS], F32, tag="pst")
        for kc in range(4):
            nc.tensor.transpose(
                pstr[:, kc * GROWS:(kc + 1) * GROWS],
                rh[:, kc * 128:(kc + 1) * 128], ident[0:GROWS, 0:GROWS])
        # group coefficient rows: conT cols {4r + 2g + i}
        cgr = []
        for e in range(2):
            cg1 = small.tile([1, GROWS], F32, tag=f"cg1_{e}")
            nc.sync.dma_start(
                cg1[:],
                t["conT"][e:e + 1, :].rearrange("o (r i) -> o r i", i=BL)[:, :, g],
            )
            r = small.tile([128, GROWS], F32, tag=f"cgr{e}")
            nc.gpsimd.partition_broadcast(r[:], cg1[:])
            cgr.append(r)
        rhT_s = []
        for e in range(2):
            x = stage.tile([128, 4 * GROWS], BF16, tag="bsmall", bufs=4)
            nc.vector.tensor_tensor(
                out=x[:].rearrange("p (k r) -> p k r", k=4),
                in0=pstr[:].rearrange("p (k r) -> p k r", k=4),
                in1=cgr[e][:].rearrange("p (o r) -> p o r", o=1).broadcast_to((128, 4, GROWS)),
                op=OP.mult,
            )
            rhT_s.append(x)
        psm = ps_lin.tile([GROWS, D], F32, tag="pslin")
        for half in range(2):
            for e in range(2):
                for kc in range(4):
                    wt = wp.tile([128, HD], BF16, tag="wchunk")
                    nc.sync.dma_start(
                        wt[:], t["wm"][e, kc * 128:(kc + 1) * 128,
                                       half * HD:(half + 1) * HD])
                    nc.tensor.matmul(
                        psm[:, half * HD:(half + 1) * HD],
                        rhT_s[e][:, kc * GROWS:(kc + 1) * GROWS],
                        wt[:],
                        start=(e == 0 and kc == 0), stop=(e == 1 and kc == 3),
                    )
        bm = stage.tile([GROWS, D], F32, tag="g16", bufs=3)
        nc.sync.dma_start(
            bm[:], t["b_m"][:].rearrange("(r i) d -> r i d", i=BL)[:, g, :])
        mst = stage.tile([GROWS, D], F32, tag="g16", bufs=3)
        nc.vector.tensor_tensor(out=mst[:], in0=psm[:], in1=bm[:], op=OP.add)
        nc.sync.dma_start(t[f"mg_in{g}"][:], mst[:])
        collective("AllReduce", t[f"mg_in{g}"][:], t[f"mg_out{g}"][:])

        x = stage.tile([GROWS, D], F32, tag="respg", bufs=1)
        nc.sync.dma_start(x[:], t[f"mg_out{g}"][:])
        qg = stage.tile([GROWS, D], F32, tag="g16", bufs=3)
        nc.sync.dma_start(
            qg[:], t["query"][:].rearrange("(r i) d -> r i d", i=BL)[:, g, :])
        # layernorm(x + qg)
        nc.vector.tensor_tensor(out=x[:], in0=x[:], in1=qg[:], op=OP.add)
        mu = small.tile([GROWS, 1], F32, tag="mu")
        nc.vector.tensor_reduce(out=mu[:], in_=x[:], axis=AX.X, op=OP.add)
        nc.vector.tensor_scalar(out=mu[:], in0=mu[:], scalar1=1.0 / D, scalar2=None,
                                op0=OP.mult)
        nc.vector.tensor_scalar(out=x[:], in0=x[:], scalar1=mu[:], scalar2=None,
                                op0=OP.subtract)
        sq = stage.tile([GROWS, D], F32, tag="g16", bufs=3)
        ssq = small.tile([GROWS, 1], F32, tag="ssq")
        nc.scalar.activation(sq[:], x[:], AF.Square, accum_out=ssq[:])
        nc.vector.tensor_scalar(out=ssq[:], in0=ssq[:], scalar1=float(D) * 1e-5,
                                scalar2=None, op0=OP.add)
        sstd = small.tile([GROWS, 1], F32, tag="sstd")
        nc.scalar.activation(sstd[:], ssq[:], AF.Sqrt)
        rstd = small.tile([GROWS, 1], F32, tag="rstd")
        nc.vector.reciprocal(rstd[:], sstd[:])
        nc.vector.tensor_scalar(out=x[:], in0=x[:], scalar1=rstd[:],
                                scalar2=float(np.sqrt(D)), op0=OP.mult, op1=OP.mult)
        nc.vector.tensor_tensor(out=x[:], in0=x[:], in1=lnsc_r[:], op=OP.mult)
        nc.vector.tensor_tensor(out=x[:], in0=x[:], in1=lnbi_r[:], op=OP.add)
        nc.sync.dma_start(
            t["resp"][:].rearrange("(r i) d -> r i d", i=BL)[:, g, :], x[:])

        # st linear: stT[p, kc*16+r] = x[r, kc*128+p]
        psr2 = ps_t.tile([128, 8 * GROWS], F32, tag="pst")
        for kc in range(8):
            nc.tensor.transpose(
                psr2[:, kc * GROWS:(kc + 1) * GROWS],
                x[:, kc * 128:(kc + 1) * 128], ident[0:GROWS, 0:GROWS])
        stT_s = []
        for e in range(2):
            y = stage.tile([128, 8 * GROWS], BF16, tag="bsmall", bufs=4)
            nc.vector.tensor_tensor(
                out=y[:].rearrange("p (k r) -> p k r", k=8),
                in0=psr2[:].rearrange("p (k r) -> p k r", k=8),
                in1=cgr[e][:].rearrange("p (o r) -> p o r", o=1).broadcast_to((128, 8, GROWS)),
                op=OP.mult,
            )
            stT_s.append(y)
        pss2 = ps_lin.tile([GROWS, HD], F32, tag="pslin")
        for e in range(2):
            for kc in range(8):
                wt = wp.tile([128, HD], BF16, tag="wchunk")
                nc.sync.dma_start(wt[:], t["wws"][e, kc * 128:(kc + 1) * 128, :])
                nc.tensor.matmul(
                    pss2[:],
                    stT_s[e][:, kc * GROWS:(kc + 1) * GROWS],
                    wt[:],
                    start=(e == 0 and kc == 0), stop=(e == 1 and kc == 7),
                )
        bs = stage.tile([GROWS, HD], F32, tag="bs_sst", bufs=2)
        nc.sync.dma_start(
            bs[:], t["b_s"][:].rearrange("(r i) d -> r i d", i=BL)[:, g, :])
        sst = stage.tile([GROWS, HD], F32, tag="bs_sst", bufs=2)
        nc.vector.tensor_tensor(out=sst[:], in0=pss2[:], in1=bs[:], op=OP.add)
        nc.sync.dma_start(t[f"sg_in{g}"][:], sst[:])
        collective("ReduceScatter", t[f"sg_in{g}"][:], t[f"sg_out{g}"][:])

    # ---------------- store phase (scatter-add + writeback) ----------------
    def scatter_group(g):
        bl = g
        strow = small.tile([1, HD], F32, tag="strow", bufs=2, name=f"strow{g}")
        nc.sync.dma_start(strow[:], t[f"sg_out{g}"][0])
        # scale st rows by 1/sum(e_wq) per head, then broadcast to 128
        # partitions with a K=1 ones-row matmul.
        stsc = small.tile([1, HD], F32, tag="stsc", bufs=2, name=f"stsc{g}")
        nc.vector.tensor_tensor(
            out=stsc[:].rearrange("o (j k d) -> o j k d", j=NPAIR, k=2),
            in0=strow[:].rearrange("o (j k d) -> o j k d", j=NPAIR, k=2),
            in1=inv_g[g][0:1, :].rearrange("o (j k u) -> o j k u", j=NPAIR, u=1)[:, :, 2:4, :]
                .broadcast_to((1, NPAIR, 2, DM)),
            op=OP.mult,
        )
        ps_st = ps_s.tile([128, HD], F32, tag="ps_strep", bufs=1, name=f"psstrep{g}")
        nc.tensor.matmul(ps_st[:], onesrow_t[:], stsc[:], start=True, stop=True)
        for h in range(H):
            j, par = h // 2, h % 2
            en = e_nat[(bl, j)]
            tmp = scat.tile([128, T * DM], F32, tag="tmp", name=f"tmp{g}_{h}")
            nc.vector.tensor_tensor(
                out=tmp[:].rearrange("p (tt d) -> p tt d", tt=T),
                in0=en[:].rearrange("p (tt k) -> p tt k", tt=T)[:, :, 2 + par:3 + par]
                    .broadcast_to((128, T, DM)),
                in1=ps_st[:, h * DM:(h + 1) * DM].rearrange("p (o d) -> p o d", o=1)
                    .broadcast_to((128, T, DM)),
                op=OP.mult,
            )
            nc.vector.tensor_tensor(
                out=tmp[:], in0=mem_sl[(bl, h)][:], in1=tmp[:], op=OP.add)
            nc.sync.dma_start(
                t["newmem"][bl, h].rearrange("(p tt) d -> p tt d", tt=T),
                tmp[:].rearrange("p (tt d) -> p tt d", tt=T),
            )

    # ---------------- schedule ----------------
    express_group(0)
    express_group(1)
    tail_group(0)
    express_group(2)
    tail_group(1)
    scatter_group(0)
    express_group(3)
    tail_group(2)
    scatter_group(1)
    tail_group(3)
    scatter_group(2)
    scatter_group(3)


def _build():
    if "nc" in _CACHE:
        return _CACHE["nc"], _CACHE["t"]
    nc = bacc.Bacc("TRN2", target_bir_lowering=False, debug=False,
                   num_devices=N_CORES)
    t = _declare(nc)
    with tile.TileContext(nc) as tc:
        _emit(tc, t)
    nc.compile()
    _CACHE["nc"] = nc
    _CACHE["t"] = t
    return nc, t


# --------------------------------------------------------------------------
# host side
# --------------------------------------------------------------------------

def _prep_in_maps(inputs):
    mem = np.asarray(inputs["memories"], dtype=np.float32)
    query = np.asarray(inputs["query"], dtype=np.float32)
    sel = np.asarray(inputs["sel_index"])
    probs = np.asarray(inputs["sel_probs"], dtype=np.float32)

    c = np.zeros((B, 16), np.float32)
    for k in range(sel.shape[1]):
        np.add.at(c, (np.arange(B), sel[:, k]), probs[:, k])

    # memt column c = tt*128 + p holds m = p*16 + tt (matches the contiguous
    # natural-layout bijection used on device)
    memt = np.ascontiguousarray(
        mem.transpose(0, 1, 3, 2).reshape(B, H, DM, 128, T).transpose(0, 1, 2, 4, 3)
        .reshape(B, H, DM, M)).astype(BF16_NP)
    wr = (np.asarray(inputs["W_read"], np.float32) * 0.125).astype(BF16_NP)
    wwq = (np.asarray(inputs["W_wq"], np.float32) * 0.125).astype(BF16_NP)
    wm = np.asarray(inputs["W_merge"], np.float32).astype(BF16_NP)
    wws = np.asarray(inputs["W_ws"], np.float32).astype(BF16_NP)

    b_qwq = np.concatenate([
        (c @ np.asarray(inputs["b_read"], np.float32)) * 0.125,
        (c @ np.asarray(inputs["b_wq"], np.float32)) * 0.125,
    ], axis=1).astype(np.float32)
    b_m = (c @ np.asarray(inputs["b_merge"], np.float32)).astype(np.float32)
    b_s = (c @ np.asarray(inputs["b_ws"], np.float32)).astype(np.float32)
    zq = np.zeros_like(b_qwq)
    zm = np.zeros_like(b_m)
    zs = np.zeros_like(b_s)

    queryT = np.ascontiguousarray(query.T)
    lnsc = np.asarray(inputs["ln_scale"], np.float32).reshape(1, D)
    lnbi = np.asarray(inputs["ln_bias"], np.float32).reshape(1, D)
    ident = np.eye(128, dtype=np.float32)
    ones = np.ones((128, 1), dtype=np.float32)
    onesrow = np.ones((1, 128), dtype=np.float32)

    in_maps = []
    for core in range(N_CORES):
        bs_ = slice(BL * core, BL * (core + 1))
        bk = slice(2 * core, 2 * core + 2)
        in_maps.append({
            "mem": np.ascontiguousarray(mem[bs_]),
            "memt": np.ascontiguousarray(memt[bs_]),
            "query": query,
            "queryT": queryT,
            "conT": np.ascontiguousarray(c[:, bk].T),
            "wr": np.ascontiguousarray(wr[bk]),
            "wwq": np.ascontiguousarray(wwq[bk]),
            "wm": np.ascontiguousarray(wm[bk]),
            "wws": np.ascontiguousarray(wws[bk]),
            "b_qwq": b_qwq if core == 0 else zq,
            "b_m": b_m if core == 0 else zm,
            "b_s": b_s if core == 0 else zs,
            "lnsc": lnsc,
            "lnbi": lnbi,
            "ident": ident,
            "ones": ones,
            "onesrow": onesrow,
        })
    return in_maps


def _assemble(results):
    response = results[0]["resp"]
    new_mem = np.concatenate([results[c]["newmem"] for c in range(N_CORES)], axis=0)
    return response, new_mem


def kernel(**inputs):
    from concourse.bass_utils import run_bass_kernel_spmd
    nc, _ = _build()
    in_maps = _prep_in_maps(inputs)
    res = run_bass_kernel_spmd(nc, in_maps, list(range(N_CORES)))
    return _assemble(res.results)


# --------------------------------------------------------------------------
# profiling helper (not used by the grading path)
# --------------------------------------------------------------------------

def _register_ntff_hook():
    if "antenv.axon_hooks" in sys.modules:
        return
    holder = {"h": None}
    mod = types.ModuleType("antenv.axon_hooks")
    mod.set_axon_ntff_profile_hook = lambda h: holder.__setitem__("h", h)
    mod.get_axon_ntff_profile_hook = lambda: holder["h"]
    sys.modules["antenv.axon_hooks"] = mod
    try:
        lib = ctypes.CDLL("/opt/axon/libaxon_pjrt.so")
        lib.axon_start_nrt_profile.argtypes = [ctypes.POINTER(ctypes.c_int64), ctypes.c_size_t]
        lib.axon_start_nrt_profile.restype = ctypes.c_int64
        lib.axon_stop_nrt_profile.argtypes = [ctypes.c_char_p]
        lib.axon_stop_nrt_profile.restype = ctypes.c_int64
    except OSError:
        return

    @contextlib.contextmanager
    def _hook(output_dir, device_ids):
        import jax
        jax.devices()
        if device_ids:
            ids = (ctypes.c_int64 * len(device_ids))(*device_ids)
            rc = lib.axon_start_nrt_profile(ids, len(device_ids))
        else:
            rc = lib.axon_start_nrt_profile(None, 0)
        if rc != 0:
            raise RuntimeError(f"axon_start_nrt_profile rc={rc}")
        try:
            yield
        finally:
            n = lib.axon_stop_nrt_profile(str(output_dir).encode())
            print(f"profile: {n} file(s) written to {output_dir}", file=sys.stderr)

    mod.set_axon_ntff_profile_hook(_hook)


def kernel_profiled(tmpdir, **inputs):
    import concourse.bass_utils as bass_utils
    _register_ntff_hook()
    bass_utils.upload_artifacts = lambda d: f"local://{d}"
    nc, _ = _build()
    in_maps = _prep_in_maps(inputs)
    res = bass_utils.run_bass_kernel_spmd(
        nc, in_maps, list(range(N_CORES)), trace=True, tmpdir=tmpdir)
    return _assemble(res.results), res.exec_time_ns
